# revision 15
# baseline (speedup 1.0000x reference)
"""Trainium2 Bass kernel: Wan-style attention block (QKV proj + QK-RMSNorm +
interleaved RoPE + softmax attention + output proj), sharded over 8 NeuronCores.

Sharding: head-parallel (16 heads -> 2 per core) for QKV + attention, with a
16KB AllReduce for the global RMSNorm statistics, then a 1MB AllToAll that
switches to sequence-parallel for the output projection (each core produces
256 exact rows of the final output).

Self-contained: hardcodes all shapes; host-side prep does transposes / bf16
conversion / pre-tiling into partition-contiguous DMA blocks / an even-odd
head-dim permutation that makes RoPE slices contiguous.
"""

import os
import sys

import numpy as np

for _p in ("/opt/trn_rl_repo", "/root/.axon_site/_ro/trn_rl_repo"):
    if _p not in sys.path and os.path.isdir(_p):
        sys.path.append(_p)

import ml_dtypes  # noqa: E402

import concourse.bass as bass  # noqa: E402
import concourse.mybir as mybir  # noqa: E402
import concourse.tile as tile  # noqa: E402
from concourse import bacc, bass_utils  # noqa: E402

BF16 = mybir.dt.bfloat16
F32 = mybir.dt.float32
AF = mybir.ActivationFunctionType
ALU = mybir.AluOpType

N_CORES = 8
P = 128
DIM = 2048
D = 128
H = 16
H_LOC = H // N_CORES          # 2 heads per core
E_LOC = H_LOC * D             # 256 local head-dims
E2 = 2 * E_LOC                # q|k concatenated
KO = DIM // P                 # 16 contraction chunks
MO = DIM // 512               # out-proj m macro-tiles
EPS = 1e-6


def build_kernel(S: int, it: int | None = None):
    """Build + compile the 8-core SPMD graph for sequence length S."""
    SC = S // P                      # s-chunks
    IT = it or min(512, S)           # i-tile (query block, == PSUM free dim)
    NI = S // IT
    S_LOC = S // N_CORES
    SH = max(1, S_LOC // P)          # out-proj s-blocks (2 at full size)
    PS = min(P, S_LOC)               # out-proj partition size

    nc = bacc.Bacc("TRN2", target_bir_lowering=False, debug=False,
                   num_devices=N_CORES, enable_asserts=False)

    def din(name, shape, dt):
        return nc.dram_tensor(name, shape, dt, kind="ExternalInput").ap()

    # all big tensors host-pre-tiled to [partition=128, contiguous free]
    xtl = din("xtl", [SC, P, KO * P], BF16)       # x chunks  [t][ki][ko*sj]
    wqkt = din("wqkt", [P, KO * E2], BF16)        # q|k weights [ki][ko*e]
    wvt = din("wvt", [P, KO * E_LOC], BF16)
    cstl = din("cstl", [P, SC * (D // 2)], F32)   # cos(even) [p][t*j]
    sntl = din("sntl", [P, SC * (D // 2)], F32)   # sin(odd)
    brqk = din("brqk", [P, E2], F32)              # bias replicated over partitions
    brv = din("brv", [P, E_LOC], F32)
    wrqk = din("wrqk", [P, E2], F32)              # norm weight replicated
    wotl = din("wotl", [MO, P, KO * 512], BF16)   # w_o.T tiled [mo][ki][ko*m]
    bor = din("bor", [1, DIM], BF16)              # b_o row
    out_loc = nc.dram_tensor("out_loc", [S_LOC, DIM], F32,
                             kind="ExternalOutput").ap()

    rg = [list(range(N_CORES))]

    with tile.TileContext(nc) as tc:
        cp = tc.alloc_tile_pool(name="const", bufs=1)
        dp = tc.alloc_tile_pool(name="dram", bufs=1, space="DRAM")
        qkp = tc.alloc_tile_pool(name="qkp", bufs=1)

        # ---- resident loads (each one contiguous DMA) ----
        wqk_sb = cp.tile([P, KO, E2], BF16, name="wqk_sb")
        wv_sb = cp.tile([P, KO, E_LOC], BF16, name="wv_sb")
        nc.sync.dma_start(wqk_sb[:], wqkt.rearrange("p (ko e) -> p ko e", ko=KO))
        nc.sync.dma_start(wv_sb[:], wvt.rearrange("p (ko e) -> p ko e", ko=KO))
        cs_sb = cp.tile([P, SC, D // 2], F32, name="cs_sb")
        sn_sb = cp.tile([P, SC, D // 2], F32, name="sn_sb")
        nc.sync.dma_start(cs_sb[:], cstl.rearrange("p (t j) -> p t j", t=SC))
        nc.sync.dma_start(sn_sb[:], sntl.rearrange("p (t j) -> p t j", t=SC))
        brqk_sb = cp.tile([P, E2], F32, name="brqk_sb")
        brv_sb = cp.tile([P, E_LOC], F32, name="brv_sb")
        wrqk_sb = cp.tile([P, E2], F32, name="wrqk_sb")
        for t_, s_ in ((brqk, brqk_sb), (brv, brv_sb), (wrqk, wrqk_sb)):
            nc.sync.dma_start(s_[:], t_)
        bor_sb = cp.tile([1, DIM], BF16, name="bor_sb")
        nc.sync.dma_start(bor_sb[:], bor)
        ones_col = cp.tile([P, 1], BF16, name="ones_col")
        nc.gpsimd.memset(ones_col[:], 1.0)
        ones_row = cp.tile([1, P], BF16, name="ones_row")
        nc.gpsimd.memset(ones_row[:], 1.0)

        # ---- persistent activations ----
        qkh_sb = qkp.tile([P, SC, E2], F32, name="qkh_sb")   # w*(x@W+b) pre-scale
        v_sb = cp.tile([P, SC, E_LOC], BF16, name="v_sb")
        ss_q = cp.tile([P, SC], F32, name="ss_q")            # sum((q+b)^2) partial
        ss_k = cp.tile([P, SC], F32, name="ss_k")
        qt = cp.tile([P, H_LOC, S], BF16, name="qt")         # [d', h, s] post-rope
        kt = cp.tile([P, H_LOC, S], BF16, name="kt")

        # ======== Stage A: QKV projections ========
        with tc.tile_pool(name="xp", bufs=3) as xp, \
             tc.tile_pool(name="psA", bufs=3, space="PSUM") as psA, \
             tc.tile_pool(name="tpA", bufs=3) as tpA:
            for t in range(SC):
                xt_t = xp.tile([P, KO, P], BF16, tag="xt")
                nc.sync.dma_start(
                    xt_t[:], xtl[t].rearrange("p (ko s) -> p ko s", ko=KO))
                # q|k fused projection
                ps_qk = psA.tile([P, E2], F32, tag="psqk")
                for ko in range(KO):
                    nc.tensor.matmul(ps_qk[:], xt_t[:, ko], wqk_sb[:, ko],
                                     start=(ko == 0), stop=(ko == KO - 1))
                tmp = tpA.tile([P, E2], F32, tag="tmp")
                nc.vector.tensor_add(tmp[:], ps_qk[:], brqk_sb[:])
                nc.vector.tensor_tensor(qkh_sb[:, t], tmp[:], wrqk_sb[:], ALU.mult)
                junk = tpA.tile([P, E_LOC], F32, tag="junk")
                nc.vector.scalar_tensor_tensor(
                    junk[:], tmp[:, :E_LOC], 1.0, tmp[:, :E_LOC], ALU.mult,
                    ALU.mult, accum_out=ss_q[:, t:t + 1])
                junk2 = tpA.tile([P, E_LOC], F32, tag="junk")
                nc.vector.scalar_tensor_tensor(
                    junk2[:], tmp[:, E_LOC:], 1.0, tmp[:, E_LOC:], ALU.mult,
                    ALU.mult, accum_out=ss_k[:, t:t + 1])
                # K rope + transpose for this chunk, streamed inside stage A
                # (K is roped unscaled; its norm scale folds into the Exp)
                rok = tpA.tile([P, E_LOC], BF16, tag="rok")
                for h in range(H_LOC):
                    b = E_LOC + h * D
                    ke = qkh_sb[:, t, b:b + 64]
                    kod = qkh_sb[:, t, b + 64:b + D]
                    u1 = tpA.tile([P, 64], F32, tag="ku1")
                    u2 = tpA.tile([P, 64], F32, tag="ku2")
                    nc.vector.tensor_tensor(u1[:], ke, cs_sb[:, t], ALU.mult)
                    nc.vector.tensor_tensor(u2[:], kod, sn_sb[:, t], ALU.mult)
                    nc.vector.tensor_tensor(rok[:, h * D:h * D + 64],
                                            u1[:], u2[:], ALU.subtract)
                    u3 = tpA.tile([P, 64], F32, tag="ku1")
                    u4 = tpA.tile([P, 64], F32, tag="ku2")
                    nc.vector.tensor_tensor(u3[:], ke, sn_sb[:, t], ALU.mult)
                    nc.vector.tensor_tensor(u4[:], kod, cs_sb[:, t], ALU.mult)
                    nc.vector.tensor_tensor(rok[:, h * D + 64:(h + 1) * D],
                                            u3[:], u4[:], ALU.add)
                keng = nc.sync if t % 2 == 0 else nc.scalar
                keng.dma_start_transpose(kt[:, :, t * P:(t + 1) * P], rok[:])
        # ======== Stage B: AllReduce RMS stats, compute scales ========
        ar_in = dp.tile([2, P, SC], F32, name="ar_in")
        ar_out = dp.tile([2, P, SC], F32, name="ar_out")
        nc.sync.dma_start(ar_in[0], ss_q[:])
        nc.sync.dma_start(ar_in[1], ss_k[:])
        nc.gpsimd.collective_compute(
            "AllReduce", ALU.add, replica_groups=rg,
            ins=[ar_in[:].opt()], outs=[ar_out[:].opt()])

        # V projection issued after the AllReduce: its matmuls keep the
        # TensorEngine busy during the collective + rope + transposes.
        with tc.tile_pool(name="xv", bufs=3) as xv, \
             tc.tile_pool(name="psV", bufs=3, space="PSUM") as psV:
            for t in range(SC):
                xv_t = xv.tile([P, KO, P], BF16, tag="xv")
                nc.sync.dma_start(
                    xv_t[:], xtl[t].rearrange("p (ko s) -> p ko s", ko=KO))
                ps_v = psV.tile([P, E_LOC], F32, tag="psv")
                for ko in range(KO):
                    nc.tensor.matmul(ps_v[:], xv_t[:, ko], wv_sb[:, ko],
                                     start=(ko == 0), stop=(ko == KO - 1))
                nc.vector.tensor_add(v_sb[:, t], ps_v[:], brv_sb[:])
        ssg_q = cp.tile([P, SC], F32, name="ssg_q")
        ssg_k = cp.tile([P, SC], F32, name="ssg_k")
        nc.sync.dma_start(ssg_q[:], ar_out[0])
        nc.sync.dma_start(ssg_k[:], ar_out[1])
        # S_q = 1/sqrt(D*(var+eps)) = rsqrt(var+eps)/sqrt(D); S_k = rsqrt(var+eps)
        tq = cp.tile([P, SC], F32, name="tq")
        tk = cp.tile([P, SC], F32, name="tk")
        eps_q = cp.tile([P, 1], F32, name="eps_q")
        eps_k = cp.tile([P, 1], F32, name="eps_k")
        nc.gpsimd.memset(eps_q[:], float(D) * EPS)
        nc.gpsimd.memset(eps_k[:], EPS)
        nc.scalar.activation(tq[:], ssg_q[:], AF.Sqrt,
                             scale=float(D) / float(DIM), bias=eps_q[:])
        nc.scalar.activation(tk[:], ssg_k[:], AF.Sqrt,
                             scale=1.0 / float(DIM), bias=eps_k[:])
        sq_sc = cp.tile([P, SC], F32, name="sq_sc")
        sk_sc = cp.tile([P, SC], F32, name="sk_sc")
        nc.vector.reciprocal(sq_sc[:], tq[:])
        nc.vector.reciprocal(sk_sc[:], tk[:])

        # ======== Stage C: Q scale + RoPE in two half-batches ========
        with tc.tile_pool(name="rp", bufs=2) as rp:
            NB = 2 if SC >= 2 else 1
            QB = SC // NB
            for bi in range(NB):
                csl = slice(bi * QB, (bi + 1) * QB)
                qs = rp.tile([P, QB, E_LOC], F32, tag="qs")
                nc.vector.tensor_tensor(
                    qs[:], qkh_sb[:, csl, :E_LOC],
                    sq_sc[:, csl, None].to_broadcast([P, QB, E_LOC]), ALU.mult)
                ro = rp.tile([P, QB, E_LOC], BF16, tag="ro")
                for h in range(H_LOC):
                    b = h * D
                    qe = qs[:, :, b:b + 64]
                    qo = qs[:, :, b + 64:b + D]
                    u1 = rp.tile([P, QB, 64], F32, tag="u1")
                    u2 = rp.tile([P, QB, 64], F32, tag="u2")
                    nc.vector.tensor_tensor(u1[:], qe, cs_sb[:, csl], ALU.mult)
                    nc.vector.tensor_tensor(u2[:], qo, sn_sb[:, csl], ALU.mult)
                    nc.vector.tensor_tensor(ro[:, :, b:b + 64], u1[:], u2[:],
                                            ALU.subtract)
                    u3 = rp.tile([P, QB, 64], F32, tag="u1")
                    u4 = rp.tile([P, QB, 64], F32, tag="u2")
                    nc.vector.tensor_tensor(u3[:], qe, sn_sb[:, csl], ALU.mult)
                    nc.vector.tensor_tensor(u4[:], qo, cs_sb[:, csl], ALU.mult)
                    nc.vector.tensor_tensor(ro[:, :, b + 64:b + D], u3[:], u4[:],
                                            ALU.add)
                for tj in range(QB):
                    t = bi * QB + tj
                    qeng = nc.sync if t % 2 == 0 else nc.scalar
                    qeng.dma_start_transpose(
                        qt[:, :, t * P:(t + 1) * P], ro[:, tj, :])
        qkp.release()

        # ======== Stage E: attention per (head, i-tile) ========
        a2a_in = [dp.tile([N_CORES, P, S_LOC], BF16, name=f"a2a_in{h}")
                  for h in range(H_LOC)]
        a2a_out = [dp.tile([N_CORES, P, S_LOC], BF16, name=f"a2a_out{h}")
                   for h in range(H_LOC)]
        with tc.tile_pool(name="psC", bufs=3, space="PSUM") as psC, \
             tc.tile_pool(name="psS", bufs=2, space="PSUM") as psS, \
             tc.tile_pool(name="psO", bufs=2, space="PSUM") as psO, \
             tc.tile_pool(name="pp", bufs=4) as pp, \
             tc.tile_pool(name="op", bufs=2) as op, \
             tc.tile_pool(name="sp", bufs=2) as sp:
            for h in range(H_LOC):
                for i in range(NI):
                    isl = slice(i * IT, (i + 1) * IT)
                    sums_ps = psS.tile([1, IT], F32, tag="sums")
                    out_ps = psO.tile([P, IT], F32, tag="outp")
                    for jc in range(SC):
                        sc_ps = psC.tile([P, IT], F32, tag="sc")
                        nc.tensor.matmul(sc_ps[:], kt[:, h, jc * P:(jc + 1) * P],
                                         qt[:, h, isl], start=True, stop=True)
                        pt = pp.tile([P, IT], BF16, tag="pt")
                        nc.scalar.activation(pt[:], sc_ps[:], AF.Exp,
                                             scale=sk_sc[:, jc:jc + 1])
                        nc.tensor.matmul(sums_ps[:], ones_col[:], pt[:],
                                         start=(jc == 0), stop=(jc == SC - 1))
                        nc.tensor.matmul(out_ps[:], v_sb[:, jc, h * D:(h + 1) * D],
                                         pt[:], start=(jc == 0), stop=(jc == SC - 1))
                    rrow = sp.tile([1, IT], F32, tag="rrow")
                    nc.vector.reciprocal(rrow[:], sums_ps[:])
                    rep_sb = sp.tile([P, IT], F32, tag="rep")
                    nc.gpsimd.partition_broadcast(rep_sb[:], rrow[:])
                    o_sb = op.tile([P, IT], BF16, tag="o_sb")
                    nc.vector.tensor_tensor(o_sb[:], out_ps[:], rep_sb[:], ALU.mult)
                    for bidx in range(max(1, IT // S_LOC)):
                        dst_core = (i * IT) // S_LOC + bidx
                        nc.sync.dma_start(
                            a2a_in[h][dst_core],
                            o_sb[:, bidx * S_LOC:(bidx + 1) * S_LOC])
                # per-head AllToAll: head h exchanges while head h+1 computes
                nc.gpsimd.collective_compute(
                    "AllToAll", ALU.bypass, replica_groups=rg,
                    ins=[a2a_in[h][:].opt()], outs=[a2a_out[h][:].opt()])

        # ======== Stage G: output projection (lhsT = activations) ========
        # a2a_out[h][a][ki][s]: global e-chunk ko = a*H_LOC + h
        at_sb = cp.tile([P, KO, S_LOC], BF16, name="at_sb")
        for a in range(N_CORES):
            for h in range(H_LOC):
                nc.sync.dma_start(at_sb[:, a * H_LOC + h, :], a2a_out[h][a])
        with tc.tile_pool(name="wp", bufs=2) as wp, \
             tc.tile_pool(name="psG", bufs=3, space="PSUM") as psG, \
             tc.tile_pool(name="f0p", bufs=MO * SH) as f0p, \
             tc.tile_pool(name="fp", bufs=3) as fp:
            f0_tiles = {}
            # group 1: even e-chunks (head 0 of each source core) — ready
            # right after the first AllToAll; overlaps the second one.
            for mo in range(MO):
                wo_e = wp.tile([P, KO // 2, 512], BF16, tag="wo_e")
                nc.sync.dma_start(
                    wo_e[:],
                    wotl[mo].rearrange("p (ko m) -> p ko m", ko=KO)[:, 0::2, :])
                for sh in range(SH):
                    g1 = psG.tile([PS, 512], F32, tag="g")
                    for a in range(N_CORES):
                        nc.tensor.matmul(
                            g1[:], at_sb[:, a * H_LOC, sh * PS:(sh + 1) * PS],
                            wo_e[:, a], start=(a == 0), stop=(a == N_CORES - 1))
                    f0 = f0p.tile([PS, 512], F32, tag="f0")
                    nc.scalar.activation(f0[:], g1[:], AF.Identity)
                    f0_tiles[(mo, sh)] = f0
            # group 2: odd e-chunks + bias, combine with group 1, store
            for mo in range(MO):
                wo_o = wp.tile([P, KO // 2, 512], BF16, tag="wo_o")
                nc.sync.dma_start(
                    wo_o[:],
                    wotl[mo].rearrange("p (ko m) -> p ko m", ko=KO)[:, 1::2, :])
                for sh in range(SH):
                    g2 = psG.tile([PS, 512], F32, tag="g")
                    for a in range(N_CORES):
                        nc.tensor.matmul(
                            g2[:], at_sb[:, a * H_LOC + 1, sh * PS:(sh + 1) * PS],
                            wo_o[:, a], start=(a == 0), stop=False)
                    nc.tensor.matmul(g2[:], ones_row[:, :PS],
                                     bor_sb[:, mo * 512:(mo + 1) * 512],
                                     start=False, stop=True)
                    f_sb = fp.tile([PS, 512], F32, tag="f_sb")
                    nc.vector.tensor_add(f_sb[:], g2[:], f0_tiles[(mo, sh)][:])
                    nc.sync.dma_start(
                        out_loc[sh * PS:(sh + 1) * PS, mo * 512:(mo + 1) * 512],
                        f_sb[:])

        cp.release()
        dp.release()

    nc.compile()
    return nc


# ---------------- host-side prep ----------------

_PERM_EO = np.concatenate([np.arange(0, D, 2), np.arange(1, D, 2)])


def prep_inputs(hidden_states, freqs_cos, freqs_sin, w_q, b_q, w_k, b_k,
                w_v, b_v, w_o, b_o, norm_q_w, norm_k_w):
    """Build the 8 per-core input maps (numpy host prep)."""
    S = hidden_states.shape[1]
    SC = S // P
    bf = ml_dtypes.bfloat16
    x = np.asarray(hidden_states[0], np.float32)
    # [t][ki(d within chunk)][ko][sj]
    xtl = np.ascontiguousarray(
        x.reshape(SC, P, KO, P).transpose(0, 3, 2, 1)).astype(bf)
    xtl = xtl.reshape(SC, P, KO * P)
    cstl = np.ascontiguousarray(
        freqs_cos[0, :, 0, 0::2].reshape(SC, P, D // 2)
        .transpose(1, 0, 2)).astype(np.float32).reshape(P, SC * (D // 2))
    sntl = np.ascontiguousarray(
        freqs_sin[0, :, 0, 1::2].reshape(SC, P, D // 2)
        .transpose(1, 0, 2)).astype(np.float32).reshape(P, SC * (D // 2))
    wotl = np.ascontiguousarray(
        w_o.T.reshape(KO, P, MO, 512).transpose(2, 1, 0, 3)).astype(bf)
    wotl = wotl.reshape(MO, P, KO * 512)
    bor = np.ascontiguousarray(b_o[None, :]).astype(bf)

    def tile_w(wt):  # [DIM, E] -> [P, KO*E] pre-tiled
        E = wt.shape[1]
        return np.ascontiguousarray(
            wt.reshape(KO, P, E).transpose(1, 0, 2)).astype(bf).reshape(P, KO * E)

    in_maps = []
    for r in range(N_CORES):
        heads = [H_LOC * r + j for j in range(H_LOC)]
        sel_qk = np.concatenate([h * D + _PERM_EO for h in heads])
        sel_v = np.concatenate([h * D + np.arange(D) for h in heads])
        wqk_t = np.concatenate([w_q[sel_qk, :].T, w_k[sel_qk, :].T], axis=1)
        rep = lambda v: np.ascontiguousarray(
            np.broadcast_to(v.astype(np.float32), (P, v.shape[0])))
        in_maps.append({
            "xtl": xtl,
            "wqkt": tile_w(wqk_t),
            "wvt": tile_w(w_v[sel_v, :].T),
            "cstl": cstl, "sntl": sntl,
            "brqk": rep(np.concatenate([b_q[sel_qk], b_k[sel_qk]])),
            "brv": rep(b_v[sel_v]),
            "wrqk": rep(np.concatenate([norm_q_w[sel_qk], norm_k_w[sel_qk]])),
            "wotl": wotl, "bor": bor,
        })
    return in_maps


_NC_CACHE = {}
LAST_EXEC_NS = None


def kernel(**inputs):
    global LAST_EXEC_NS
    inputs = {k: np.asarray(v) for k, v in inputs.items()}
    S = inputs["hidden_states"].shape[1]
    if S not in _NC_CACHE:
        _NC_CACHE[S] = build_kernel(S)
    nc = _NC_CACHE[S]
    in_maps = prep_inputs(**inputs)
    trace = bool(int(os.environ.get("KERNEL_TRACE", "0")))
    res = bass_utils.run_bass_kernel_spmd(
        nc, in_maps, core_ids=list(range(N_CORES)), trace=trace)
    LAST_EXEC_NS = res.exec_time_ns
    S_LOC = S // N_CORES
    out = np.empty((1, S, DIM), np.float32)
    for r in range(N_CORES):
        out[0, r * S_LOC:(r + 1) * S_LOC, :] = res.results[r]["out_loc"]
    return out


# revision 16
# speedup vs baseline: 1.0294x; 1.0294x over previous
"""Trainium2 Bass kernel: Wan-style attention block (QKV proj + QK-RMSNorm +
interleaved RoPE + softmax attention + output proj), sharded over 8 NeuronCores.

Sharding: head-parallel (16 heads -> 2 per core) for QKV + attention, with a
16KB AllReduce for the global RMSNorm statistics, then a 1MB AllToAll that
switches to sequence-parallel for the output projection (each core produces
256 exact rows of the final output).

Self-contained: hardcodes all shapes; host-side prep does transposes / bf16
conversion / pre-tiling into partition-contiguous DMA blocks / an even-odd
head-dim permutation that makes RoPE slices contiguous.
"""

import os
import sys

import numpy as np

for _p in ("/opt/trn_rl_repo", "/root/.axon_site/_ro/trn_rl_repo"):
    if _p not in sys.path and os.path.isdir(_p):
        sys.path.append(_p)

import ml_dtypes  # noqa: E402

import concourse.bass as bass  # noqa: E402
import concourse.mybir as mybir  # noqa: E402
import concourse.tile as tile  # noqa: E402
from concourse import bacc, bass_utils  # noqa: E402

BF16 = mybir.dt.bfloat16
F32 = mybir.dt.float32
AF = mybir.ActivationFunctionType
ALU = mybir.AluOpType

N_CORES = 8
P = 128
DIM = 2048
D = 128
H = 16
H_LOC = H // N_CORES          # 2 heads per core
E_LOC = H_LOC * D             # 256 local head-dims
E2 = 2 * E_LOC                # q|k concatenated
KO = DIM // P                 # 16 contraction chunks
MO = DIM // 512               # out-proj m macro-tiles
EPS = 1e-6


def build_kernel(S: int, it: int | None = None):
    """Build + compile the 8-core SPMD graph for sequence length S."""
    SC = S // P                      # s-chunks
    IT = it or min(512, S)           # i-tile (query block, == PSUM free dim)
    NI = S // IT
    S_LOC = S // N_CORES
    SH = max(1, S_LOC // P)          # out-proj s-blocks (2 at full size)
    PS = min(P, S_LOC)               # out-proj partition size

    nc = bacc.Bacc("TRN2", target_bir_lowering=False, debug=False,
                   num_devices=N_CORES, enable_asserts=False)

    def din(name, shape, dt):
        return nc.dram_tensor(name, shape, dt, kind="ExternalInput").ap()

    # all big tensors host-pre-tiled to [partition=128, contiguous free]
    xtl = din("xtl", [SC, P, KO * P], BF16)       # x chunks  [t][ki][ko*sj]
    wqkt = din("wqkt", [P, KO * E2], BF16)        # q|k weights [ki][ko*e]
    wvt = din("wvt", [P, KO * E_LOC], BF16)
    cstl = din("cstl", [P, SC * (D // 2)], F32)   # cos(even) [p][t*j]
    sntl = din("sntl", [P, SC * (D // 2)], F32)   # sin(odd)
    brqk = din("brqk", [P, E2], F32)              # bias replicated over partitions
    brv = din("brv", [P, E_LOC], F32)
    wrqk = din("wrqk", [P, E2], F32)              # norm weight replicated
    wotl = din("wotl", [MO, P, KO * 512], BF16)   # w_o.T tiled [mo][ki][ko*m]
    bor = din("bor", [1, DIM], BF16)              # b_o row
    out_loc = nc.dram_tensor("out_loc", [S_LOC, DIM], F32,
                             kind="ExternalOutput").ap()

    rg = [list(range(N_CORES))]

    with tile.TileContext(nc) as tc:
        cp = tc.alloc_tile_pool(name="const", bufs=1)
        dp = tc.alloc_tile_pool(name="dram", bufs=1, space="DRAM")
        qkp = tc.alloc_tile_pool(name="qkp", bufs=1)

        # ---- resident loads (each one contiguous DMA) ----
        wqk_sb = cp.tile([P, KO, E2], BF16, name="wqk_sb")
        wv_sb = cp.tile([P, KO, E_LOC], BF16, name="wv_sb")
        nc.sync.dma_start(wqk_sb[:], wqkt.rearrange("p (ko e) -> p ko e", ko=KO))
        nc.sync.dma_start(wv_sb[:], wvt.rearrange("p (ko e) -> p ko e", ko=KO))
        cs_sb = cp.tile([P, SC, D // 2], F32, name="cs_sb")
        sn_sb = cp.tile([P, SC, D // 2], F32, name="sn_sb")
        nc.sync.dma_start(cs_sb[:], cstl.rearrange("p (t j) -> p t j", t=SC))
        nc.sync.dma_start(sn_sb[:], sntl.rearrange("p (t j) -> p t j", t=SC))
        brqk_sb = cp.tile([P, E2], F32, name="brqk_sb")
        brv_sb = cp.tile([P, E_LOC], F32, name="brv_sb")
        wrqk_sb = cp.tile([P, E2], F32, name="wrqk_sb")
        for t_, s_ in ((brqk, brqk_sb), (brv, brv_sb), (wrqk, wrqk_sb)):
            nc.sync.dma_start(s_[:], t_)
        bor_sb = cp.tile([1, DIM], BF16, name="bor_sb")
        nc.sync.dma_start(bor_sb[:], bor)
        ones_col = cp.tile([P, 1], BF16, name="ones_col")
        nc.gpsimd.memset(ones_col[:], 1.0)
        ones_row = cp.tile([1, P], BF16, name="ones_row")
        nc.gpsimd.memset(ones_row[:], 1.0)

        # ---- persistent activations ----
        qkh_sb = qkp.tile([P, SC, E2], F32, name="qkh_sb")   # w*(x@W+b) pre-scale
        v_sb = cp.tile([P, SC, E_LOC], BF16, name="v_sb")
        ss_q = cp.tile([P, SC], F32, name="ss_q")            # sum((q+b)^2) partial
        ss_k = cp.tile([P, SC], F32, name="ss_k")
        qt = cp.tile([P, H_LOC, S], BF16, name="qt")         # [d', h, s] post-rope
        kt = cp.tile([P, H_LOC, S], BF16, name="kt")

        # ======== Stage A: QKV projections ========
        with tc.tile_pool(name="xp", bufs=3) as xp, \
             tc.tile_pool(name="psA", bufs=3, space="PSUM") as psA, \
             tc.tile_pool(name="tpA", bufs=3) as tpA:
            for t in range(SC):
                xt_t = xp.tile([P, KO, P], BF16, tag="xt")
                nc.sync.dma_start(
                    xt_t[:], xtl[t].rearrange("p (ko s) -> p ko s", ko=KO))
                # q|k fused projection
                ps_qk = psA.tile([P, E2], F32, tag="psqk")
                for ko in range(KO):
                    nc.tensor.matmul(ps_qk[:], xt_t[:, ko], wqk_sb[:, ko],
                                     start=(ko == 0), stop=(ko == KO - 1))
                tmp = tpA.tile([P, E2], F32, tag="tmp")
                nc.vector.tensor_add(tmp[:], ps_qk[:], brqk_sb[:])
                nc.vector.tensor_tensor(qkh_sb[:, t], tmp[:], wrqk_sb[:], ALU.mult)
                junk = tpA.tile([P, E_LOC], F32, tag="junk")
                nc.vector.scalar_tensor_tensor(
                    junk[:], tmp[:, :E_LOC], 1.0, tmp[:, :E_LOC], ALU.mult,
                    ALU.mult, accum_out=ss_q[:, t:t + 1])
                junk2 = tpA.tile([P, E_LOC], F32, tag="junk")
                nc.vector.scalar_tensor_tensor(
                    junk2[:], tmp[:, E_LOC:], 1.0, tmp[:, E_LOC:], ALU.mult,
                    ALU.mult, accum_out=ss_k[:, t:t + 1])
        # ======== Stage B: AllReduce RMS stats, compute scales ========
        ar_in = dp.tile([2, P, SC], F32, name="ar_in")
        ar_out = dp.tile([2, P, SC], F32, name="ar_out")
        nc.sync.dma_start(ar_in[0], ss_q[:])
        nc.sync.dma_start(ar_in[1], ss_k[:])
        nc.gpsimd.collective_compute(
            "AllReduce", ALU.add, replica_groups=rg,
            ins=[ar_in[:].opt()], outs=[ar_out[:].opt()])

        # V projection issued after the AllReduce: its matmuls keep the
        # TensorEngine busy during the collective + rope + transposes.
        with tc.tile_pool(name="xv", bufs=3) as xv, \
             tc.tile_pool(name="psV", bufs=3, space="PSUM") as psV:
            for t in range(SC):
                xv_t = xv.tile([P, KO, P], BF16, tag="xv")
                nc.sync.dma_start(
                    xv_t[:], xtl[t].rearrange("p (ko s) -> p ko s", ko=KO))
                ps_v = psV.tile([P, E_LOC], F32, tag="psv")
                for ko in range(KO):
                    nc.tensor.matmul(ps_v[:], xv_t[:, ko], wv_sb[:, ko],
                                     start=(ko == 0), stop=(ko == KO - 1))
                nc.vector.tensor_add(v_sb[:, t], ps_v[:], brv_sb[:])
        ssg_q = cp.tile([P, SC], F32, name="ssg_q")
        ssg_k = cp.tile([P, SC], F32, name="ssg_k")
        nc.sync.dma_start(ssg_q[:], ar_out[0])
        nc.sync.dma_start(ssg_k[:], ar_out[1])
        # S_q = 1/sqrt(D*(var+eps)) = rsqrt(var+eps)/sqrt(D); S_k = rsqrt(var+eps)
        tq = cp.tile([P, SC], F32, name="tq")
        tk = cp.tile([P, SC], F32, name="tk")
        eps_q = cp.tile([P, 1], F32, name="eps_q")
        eps_k = cp.tile([P, 1], F32, name="eps_k")
        nc.gpsimd.memset(eps_q[:], float(D) * EPS)
        nc.gpsimd.memset(eps_k[:], EPS)
        nc.scalar.activation(tq[:], ssg_q[:], AF.Sqrt,
                             scale=float(D) / float(DIM), bias=eps_q[:])
        nc.scalar.activation(tk[:], ssg_k[:], AF.Sqrt,
                             scale=1.0 / float(DIM), bias=eps_k[:])
        sq_sc = cp.tile([P, SC], F32, name="sq_sc")
        sk_sc = cp.tile([P, SC], F32, name="sk_sc")
        nc.vector.reciprocal(sq_sc[:], tq[:])
        nc.vector.reciprocal(sk_sc[:], tk[:])

        # ======== Stage C: RoPE in half-batches; K first (no AR dep) ========
        with tc.tile_pool(name="rp", bufs=2) as rp:
            NB = 2 if SC >= 2 else 1
            QB = SC // NB
            for which, bi in [("k", b) for b in range(NB)] + \
                             [("q", b) for b in range(NB)]:
                csl = slice(bi * QB, (bi + 1) * QB)
                if which == "k":
                    qs = qkh_sb[:, csl, E_LOC:]
                    dst = kt
                else:
                    qs = rp.tile([P, QB, E_LOC], F32, tag="qs")
                    nc.vector.tensor_tensor(
                        qs[:], qkh_sb[:, csl, :E_LOC],
                        sq_sc[:, csl, None].to_broadcast([P, QB, E_LOC]),
                        ALU.mult)
                    dst = qt
                ro = rp.tile([P, QB, E_LOC], BF16, tag="ro")
                for h in range(H_LOC):
                    b = h * D
                    qe = qs[:, :, b:b + 64]
                    qo = qs[:, :, b + 64:b + D]
                    u1 = rp.tile([P, QB, 64], F32, tag="u1")
                    u2 = rp.tile([P, QB, 64], F32, tag="u2")
                    nc.vector.tensor_tensor(u1[:], qe, cs_sb[:, csl], ALU.mult)
                    nc.vector.tensor_tensor(u2[:], qo, sn_sb[:, csl], ALU.mult)
                    nc.vector.tensor_tensor(ro[:, :, b:b + 64], u1[:], u2[:],
                                            ALU.subtract)
                    u3 = rp.tile([P, QB, 64], F32, tag="u1")
                    u4 = rp.tile([P, QB, 64], F32, tag="u2")
                    nc.vector.tensor_tensor(u3[:], qe, sn_sb[:, csl], ALU.mult)
                    nc.vector.tensor_tensor(u4[:], qo, cs_sb[:, csl], ALU.mult)
                    nc.vector.tensor_tensor(ro[:, :, b + 64:b + D], u3[:], u4[:],
                                            ALU.add)
                for tj in range(QB):
                    t = bi * QB + tj
                    qeng = nc.sync if t % 2 == 0 else nc.scalar
                    qeng.dma_start_transpose(
                        dst[:, :, t * P:(t + 1) * P], ro[:, tj, :])
        qkp.release()

        # ======== Stage E: attention per (head, i-tile) ========
        a2a_in = [dp.tile([N_CORES, P, S_LOC], BF16, name=f"a2a_in{h}")
                  for h in range(H_LOC)]
        a2a_out = [dp.tile([N_CORES, P, S_LOC], BF16, name=f"a2a_out{h}")
                   for h in range(H_LOC)]
        with tc.tile_pool(name="psC", bufs=3, space="PSUM") as psC, \
             tc.tile_pool(name="psS", bufs=2, space="PSUM") as psS, \
             tc.tile_pool(name="psO", bufs=2, space="PSUM") as psO, \
             tc.tile_pool(name="pp", bufs=4) as pp, \
             tc.tile_pool(name="op", bufs=2) as op, \
             tc.tile_pool(name="sp", bufs=2) as sp:
            for h in range(H_LOC):
                for i in range(NI):
                    isl = slice(i * IT, (i + 1) * IT)
                    sums_ps = psS.tile([1, IT], F32, tag="sums")
                    out_ps = psO.tile([P, IT], F32, tag="outp")
                    for jc in range(SC):
                        sc_ps = psC.tile([P, IT], F32, tag="sc")
                        nc.tensor.matmul(sc_ps[:], kt[:, h, jc * P:(jc + 1) * P],
                                         qt[:, h, isl], start=True, stop=True)
                        pt = pp.tile([P, IT], BF16, tag="pt")
                        nc.scalar.activation(pt[:], sc_ps[:], AF.Exp,
                                             scale=sk_sc[:, jc:jc + 1])
                        nc.tensor.matmul(sums_ps[:], ones_col[:], pt[:],
                                         start=(jc == 0), stop=(jc == SC - 1))
                        nc.tensor.matmul(out_ps[:], v_sb[:, jc, h * D:(h + 1) * D],
                                         pt[:], start=(jc == 0), stop=(jc == SC - 1))
                    rrow = sp.tile([1, IT], F32, tag="rrow")
                    nc.vector.reciprocal(rrow[:], sums_ps[:])
                    rep_sb = sp.tile([P, IT], F32, tag="rep")
                    nc.gpsimd.partition_broadcast(rep_sb[:], rrow[:])
                    o_sb = op.tile([P, IT], BF16, tag="o_sb")
                    nc.vector.tensor_tensor(o_sb[:], out_ps[:], rep_sb[:], ALU.mult)
                    for bidx in range(max(1, IT // S_LOC)):
                        dst_core = (i * IT) // S_LOC + bidx
                        nc.sync.dma_start(
                            a2a_in[h][dst_core],
                            o_sb[:, bidx * S_LOC:(bidx + 1) * S_LOC])
                # per-head AllToAll: head h exchanges while head h+1 computes
                nc.gpsimd.collective_compute(
                    "AllToAll", ALU.bypass, replica_groups=rg,
                    ins=[a2a_in[h][:].opt()], outs=[a2a_out[h][:].opt()])

        # ======== Stage G: output projection (lhsT = activations) ========
        # a2a_out[h][a][ki][s]: global e-chunk ko = a*H_LOC + h
        at_sb = cp.tile([P, KO, S_LOC], BF16, name="at_sb")
        for a in range(N_CORES):
            for h in range(H_LOC):
                nc.sync.dma_start(at_sb[:, a * H_LOC + h, :], a2a_out[h][a])
        with tc.tile_pool(name="wp", bufs=2) as wp, \
             tc.tile_pool(name="psG", bufs=3, space="PSUM") as psG, \
             tc.tile_pool(name="f0p", bufs=MO * SH) as f0p, \
             tc.tile_pool(name="fp", bufs=3) as fp:
            f0_tiles = {}
            # group 1: even e-chunks (head 0 of each source core) — ready
            # right after the first AllToAll; overlaps the second one.
            for mo in range(MO):
                wo_e = wp.tile([P, KO // 2, 512], BF16, tag="wo_e")
                nc.sync.dma_start(
                    wo_e[:],
                    wotl[mo].rearrange("p (ko m) -> p ko m", ko=KO)[:, 0::2, :])
                for sh in range(SH):
                    g1 = psG.tile([PS, 512], F32, tag="g")
                    for a in range(N_CORES):
                        nc.tensor.matmul(
                            g1[:], at_sb[:, a * H_LOC, sh * PS:(sh + 1) * PS],
                            wo_e[:, a], start=(a == 0), stop=(a == N_CORES - 1))
                    f0 = f0p.tile([PS, 512], F32, tag="f0")
                    nc.scalar.activation(f0[:], g1[:], AF.Identity)
                    f0_tiles[(mo, sh)] = f0
            # group 2: odd e-chunks + bias, combine with group 1, store
            for mo in range(MO):
                wo_o = wp.tile([P, KO // 2, 512], BF16, tag="wo_o")
                nc.sync.dma_start(
                    wo_o[:],
                    wotl[mo].rearrange("p (ko m) -> p ko m", ko=KO)[:, 1::2, :])
                for sh in range(SH):
                    g2 = psG.tile([PS, 512], F32, tag="g")
                    for a in range(N_CORES):
                        nc.tensor.matmul(
                            g2[:], at_sb[:, a * H_LOC + 1, sh * PS:(sh + 1) * PS],
                            wo_o[:, a], start=(a == 0), stop=False)
                    nc.tensor.matmul(g2[:], ones_row[:, :PS],
                                     bor_sb[:, mo * 512:(mo + 1) * 512],
                                     start=False, stop=True)
                    f_sb = fp.tile([PS, 512], F32, tag="f_sb")
                    nc.vector.tensor_add(f_sb[:], g2[:], f0_tiles[(mo, sh)][:])
                    nc.sync.dma_start(
                        out_loc[sh * PS:(sh + 1) * PS, mo * 512:(mo + 1) * 512],
                        f_sb[:])

        cp.release()
        dp.release()

    nc.compile()
    return nc


# ---------------- host-side prep ----------------

_PERM_EO = np.concatenate([np.arange(0, D, 2), np.arange(1, D, 2)])


def prep_inputs(hidden_states, freqs_cos, freqs_sin, w_q, b_q, w_k, b_k,
                w_v, b_v, w_o, b_o, norm_q_w, norm_k_w):
    """Build the 8 per-core input maps (numpy host prep)."""
    S = hidden_states.shape[1]
    SC = S // P
    bf = ml_dtypes.bfloat16
    x = np.asarray(hidden_states[0], np.float32)
    # [t][ki(d within chunk)][ko][sj]
    xtl = np.ascontiguousarray(
        x.reshape(SC, P, KO, P).transpose(0, 3, 2, 1)).astype(bf)
    xtl = xtl.reshape(SC, P, KO * P)
    cstl = np.ascontiguousarray(
        freqs_cos[0, :, 0, 0::2].reshape(SC, P, D // 2)
        .transpose(1, 0, 2)).astype(np.float32).reshape(P, SC * (D // 2))
    sntl = np.ascontiguousarray(
        freqs_sin[0, :, 0, 1::2].reshape(SC, P, D // 2)
        .transpose(1, 0, 2)).astype(np.float32).reshape(P, SC * (D // 2))
    wotl = np.ascontiguousarray(
        w_o.T.reshape(KO, P, MO, 512).transpose(2, 1, 0, 3)).astype(bf)
    wotl = wotl.reshape(MO, P, KO * 512)
    bor = np.ascontiguousarray(b_o[None, :]).astype(bf)

    def tile_w(wt):  # [DIM, E] -> [P, KO*E] pre-tiled
        E = wt.shape[1]
        return np.ascontiguousarray(
            wt.reshape(KO, P, E).transpose(1, 0, 2)).astype(bf).reshape(P, KO * E)

    in_maps = []
    for r in range(N_CORES):
        heads = [H_LOC * r + j for j in range(H_LOC)]
        sel_qk = np.concatenate([h * D + _PERM_EO for h in heads])
        sel_v = np.concatenate([h * D + np.arange(D) for h in heads])
        wqk_t = np.concatenate([w_q[sel_qk, :].T, w_k[sel_qk, :].T], axis=1)
        rep = lambda v: np.ascontiguousarray(
            np.broadcast_to(v.astype(np.float32), (P, v.shape[0])))
        in_maps.append({
            "xtl": xtl,
            "wqkt": tile_w(wqk_t),
            "wvt": tile_w(w_v[sel_v, :].T),
            "cstl": cstl, "sntl": sntl,
            "brqk": rep(np.concatenate([b_q[sel_qk], b_k[sel_qk]])),
            "brv": rep(b_v[sel_v]),
            "wrqk": rep(np.concatenate([norm_q_w[sel_qk], norm_k_w[sel_qk]])),
            "wotl": wotl, "bor": bor,
        })
    return in_maps


_NC_CACHE = {}
LAST_EXEC_NS = None


def kernel(**inputs):
    global LAST_EXEC_NS
    inputs = {k: np.asarray(v) for k, v in inputs.items()}
    S = inputs["hidden_states"].shape[1]
    if S not in _NC_CACHE:
        _NC_CACHE[S] = build_kernel(S)
    nc = _NC_CACHE[S]
    in_maps = prep_inputs(**inputs)
    trace = bool(int(os.environ.get("KERNEL_TRACE", "0")))
    res = bass_utils.run_bass_kernel_spmd(
        nc, in_maps, core_ids=list(range(N_CORES)), trace=trace)
    LAST_EXEC_NS = res.exec_time_ns
    S_LOC = S // N_CORES
    out = np.empty((1, S, DIM), np.float32)
    for r in range(N_CORES):
        out[0, r * S_LOC:(r + 1) * S_LOC, :] = res.results[r]["out_loc"]
    return out
